# revision 21
# baseline (speedup 1.0000x reference)
"""Block-quantized FP8 linear (KLinearFP8) on 8 trn2 NeuronCores.

y[m, n] = sum_k x_dq[m, k] * w_dq[n, k]
  x_dq: per-(row, 128-block) fp8e4m3fn-simulated quantization of x
  w_dq: weight (fp8 values held in fp32) * per-128x128-block scale

Sharding: column-parallel. weight/weight_scale_inv split along N across 8
cores, x replicated; each core computes y[:, c*2048:(c+1)*2048].

Per-core kernel (v2, PE-roofline oriented):
  - w is shipped from host as bf16 (exact: values are fp8-representable),
    already transposed/blocked to the k-on-partitions layout
    wT [128k, KB, NSH] (pure layout permutation, i.e. weight packing).
    On device it is a plain wide DMA into a persistent SBUF tile, in
    4-kb groups split over two queues so matmuls can start early. No
    PE-array transposes and no XBAR use on the weight path.
  - block scales are shipped pre-expanded (bf16, each value repeated
    128x -- pure replication) and staged into SBUF by partition-
    replicate DMAs (stride-0 partition dim), so the on-device scale
    multiply is a unit-stride bf16 tensor_tensor on vector (eligible
    for the DVE 2x perf mode; a stride-0 broadcast operand would force
    1x). Scales are applied to wT in place, per 2-kb group, emitted
    before any matmul that reads the group (Tile follows program
    order). gpsimd is kept to DMA triggers only: its custom-op
    library swaps cost 7-11us, and vector+gpsimd elementwise ops
    contend on SBUF ports, halving both.
  - x path per 128-row m-tile (two K-halves): quantize to TRN fp8e4
    with scale amax/224 (power-of-two rescale of the reference's
    amax/448 grid -> identical rounding), dequantize to bf16, XBAR
    transpose to k-on-partitions. ALL XBAR transposes live on the
    nc.sync queue: the XBAR is a globally exclusive resource --
    concurrent transposes on different queues corrupt each other
    (probed on HW).
  - GEMM: per m-tile, kb outer / 512-col chunk inner; each LDWEIGHTS
    (xT block) is shared by 4 N=512 bf16 matmuls accumulating fp32
    into 4 full PSUM banks; 8-bank rotation double-buffers m-tiles.
  - PSUM evacuation runs entirely on the scalar engine: evac copies
    queued on vector sit behind the next m-tile's quant chain and
    delay the PSUM bank release the next-next m-tile's first matmul
    waits on (~3us PE stall every other m-tile, seen in trace).
"""

import numpy as np

M, K, N = 4096, 4096, 16384
NCORES = 8
NSH = N // NCORES          # 2048 columns of y per core
P = 128
KB = K // P                # 32 k-blocks
KH = KB // 2               # 16 k-blocks per half
MT = M // P                # 32 m-tiles
NB = NSH // P              # 16 n-blocks per core
NCH = NSH // 512           # 4 psum chunks of 512
CHW = 512
FP8_SAFE = 224.0           # 448/2: fits TRN e4m3 (max 240), same rounding grid

_NC_CACHE = {}


def _build(M=M, K=K, NSH=NSH, debug=False):
    import concourse.bass as bass  # noqa: F401
    import concourse.mybir as mybir
    import concourse.tile as tile
    from concourse import bacc

    KB = K // P
    KH = KB // 2
    MT = M // P
    NB = NSH // P
    CHW = min(512, NSH)
    NCH = NSH // CHW
    GRP = 4 if KB % 4 == 0 else 1   # k-blocks per w-load piece
    NG = KB // GRP
    SG = 2 if KB % 2 == 0 else 1    # k-blocks per scale group
    NSG = KB // SG

    f32, bf16, f8 = mybir.dt.float32, mybir.dt.bfloat16, mybir.dt.float8e4

    nc = bacc.Bacc(None, target_bir_lowering=False, debug=debug)
    x_d = nc.declare_dram_parameter("x", [M, K], f32, isOutput=False)
    # host ships w pre-transposed/blocked: wt[k', kb*NSH + n] = w[n, kb*128+k']
    wt_d = nc.declare_dram_parameter("wt", [P, KB * NSH], bf16, isOutput=False)
    # host ships block scales bf16, expanded 32x: wse[0, (kb*NB+nb)*32+j]
    SE = 32
    wse_d = nc.declare_dram_parameter("wse", [1, KB * NB * SE], bf16,
                                      isOutput=False)
    y_d = nc.declare_dram_parameter("y", [M, NSH], f32, isOutput=True)

    with tile.TileContext(nc) as tc:
        with (
            tc.tile_pool(name="wt", bufs=1) as wtp,
            tc.tile_pool(name="sepool", bufs=2) as sepool,
            tc.tile_pool(name="xrow", bufs=2) as xrp,
            tc.tile_pool(name="xq", bufs=2) as xqp,
            tc.tile_pool(name="xdq", bufs=3) as xdp,
            tc.tile_pool(name="xtp", bufs=6) as xtp,
            tc.tile_pool(name="scales", bufs=3) as spool,
            tc.tile_pool(name="ypool", bufs=6) as ypool,
            tc.tile_pool(name="psum", bufs=8, space="PSUM") as psum,
        ):
            # ---- weight load: plain wide DMAs of the pre-packed layout,
            # ALL on the scalar queue: the queue ring drains FIFO, so the
            # pieces arrive front-to-back without flooding the other
            # queues (x rows / scale fills) the first matmuls depend on.
            wT = wtp.tile([P, KB * NSH], bf16)
            for g in range(NG):
                s = g * GRP * NSH
                e = (g + 1) * GRP * NSH
                nc.scalar.dma_start(wT[:, s:e], wt_d[:, s:e])

            SEW = SG * NB * SE

            def scale_group(g):
                # stage the 32-wide expanded scales via partition-replicate
                # DMA (tiny), then an in-place bf16 multiply on vector whose
                # innermost dims are unit-stride (DVE 2x perf mode; the
                # replicate factor rides a stride-0 MIDDLE dim, which the
                # 2x trigger does not constrain)
                se = sepool.tile([P, SG * NB, SE], bf16, tag="se")
                nc.sync.dma_start(
                    se[:].rearrange("p a b -> p (a b)"),
                    wse_d[:, g * SEW:(g + 1) * SEW].to_broadcast((P, SEW)),
                )
                view = wT[:, g * SG * NSH:(g + 1) * SG * NSH].rearrange(
                    "p (a f x) -> p a f x", a=SG * NB, x=SE
                )
                nc.vector.tensor_tensor(
                    view, view,
                    se[:, :, None, :].to_broadcast((P, SG * NB, P // SE, SE)),
                    mybir.AluOpType.mult,
                )

            def quant_half(mt, kh, splits=1):
                # splits>1 quantizes/transposes in kb-subranges (slice-level
                # ops on full-size tiles) to shorten the time to the first
                # usable xT block (startup only)
                ms = slice(mt * P, (mt + 1) * P)
                xrow = xrp.tile([P, KH, P], f32, tag="xrow")
                sc = spool.tile([P, 3, KH], f32, tag="sc")
                xq = xqp.tile([P, KH, P], f8, tag="xq")
                xdq = xdp.tile([P, KH, P], bf16, tag="xdq")
                xT = xtp.tile([P, KH, P], bf16, tag="xT")
                kq = KH // splits
                for s in range(splits):
                    q = slice(s * kq, (s + 1) * kq)
                    k0 = kh * KH + s * kq
                    ks = slice(k0 * P, (k0 + kq) * P)
                    nc.gpsimd.dma_start(
                        xrow[:, q, :],
                        x_d[ms, ks].rearrange("m (kb x) -> m kb x", x=P),
                    )
                    amax, rinv, s2 = sc[:, 0, q], sc[:, 1, q], sc[:, 2, q]
                    nc.vector.tensor_reduce(
                        amax, xrow[:, q, :], axis=mybir.AxisListType.X,
                        op=mybir.AluOpType.max, apply_absolute_value=True,
                    )
                    nc.vector.reciprocal(rinv, amax)
                    nc.vector.tensor_scalar_mul(rinv, rinv, float(FP8_SAFE))
                    nc.vector.tensor_scalar_mul(s2, amax, float(1.0 / FP8_SAFE))
                    nc.vector.tensor_tensor(
                        xq[:, q, :], xrow[:, q, :],
                        rinv[:, :, None].to_broadcast((P, kq, P)),
                        mybir.AluOpType.mult,
                    )
                    nc.vector.tensor_tensor(
                        xdq[:, q, :], xq[:, q, :],
                        s2[:, :, None].to_broadcast((P, kq, P)),
                        mybir.AluOpType.mult,
                    )
                    nc.sync.dma_start_transpose(
                        xT[:, q, :],
                        xdq[:, q, :].rearrange("p a b -> p (a b)"),
                    )
                return xT

            # Emission order (Tile semantics follow program order): first
            # two scale groups, then mt0+mt1 x pipelines (no wT dep; keeps
            # the PE fed while the remaining scales stream), then the rest
            # of the scales, then the m-loop.
            scale_group(0)
            scale_group(1)
            xT0 = [quant_half(0, 0, splits=2 if KH % 2 == 0 else 1),
                   quant_half(0, 1)]
            xT1 = [quant_half(1, 0), quant_half(1, 1)] if MT > 1 else None
            for g in range(2, NSG):
                scale_group(g)

            for mt in range(MT):
                ms = slice(mt * P, (mt + 1) * P)
                if mt == 0:
                    xThalf = xT0
                elif mt == 1:
                    xThalf = xT1
                else:
                    xThalf = [quant_half(mt, 0), quant_half(mt, 1)]

                pts = [
                    psum.tile([P, CHW], mybir.dt.float32, name=f"pt{c}", tag="pt")
                    for c in range(NCH)
                ]
                for kb in range(KB):
                    kh, kl = divmod(kb, KH)
                    for c in range(NCH):
                        nc.tensor.matmul(
                            pts[c][:],
                            xThalf[kh][:, kl, :],
                            wT[:, kb * NSH + c * CHW:kb * NSH + (c + 1) * CHW],
                            start=(kb == 0),
                            stop=(kb == KB - 1),
                        )
                for c in range(NCH):
                    yt = ypool.tile([P, CHW], mybir.dt.float32, tag="yt")
                    nc.scalar.activation(
                        yt[:], pts[c][:],
                        mybir.ActivationFunctionType.Copy,
                    )
                    nc.gpsimd.dma_start(y_d[ms, c * CHW:(c + 1) * CHW], yt[:])

    nc.compile()
    return nc


def _prep_inputs(x, weight, weight_scale_inv):
    import ml_dtypes

    x = np.ascontiguousarray(np.asarray(x, dtype=np.float32))
    weight = np.asarray(weight, dtype=np.float32)
    ws = np.asarray(weight_scale_inv, dtype=np.float32)
    KBl = weight.shape[1] // P
    nshard = weight.shape[0] // NCORES
    in_maps = []
    for c in range(NCORES):
        wsh = weight[c * nshard:(c + 1) * nshard].astype(ml_dtypes.bfloat16)
        # pack: wt[k', kb, n] = w[n, kb*128+k']  -> flat [128, KB*NSH]
        wt = np.ascontiguousarray(
            wsh.reshape(nshard, KBl, P).transpose(2, 1, 0)
        ).reshape(P, KBl * nshard)
        nbsh = nshard // P
        # expanded bf16 block scales: [kb, nb] -> repeat each value 32x
        wse = np.repeat(
            np.ascontiguousarray(
                ws[c * nbsh:(c + 1) * nbsh].T
            ).astype(ml_dtypes.bfloat16).reshape(-1),
            32,
        )[None, :]
        in_maps.append({"x": x, "wt": wt, "wse": wse})
    return in_maps


def kernel(x, weight, weight_scale_inv):
    from concourse.bass_utils import run_bass_kernel_spmd

    if "nc" not in _NC_CACHE:
        _NC_CACHE["nc"] = _build()
    nc = _NC_CACHE["nc"]

    in_maps = _prep_inputs(x, weight, weight_scale_inv)
    res = run_bass_kernel_spmd(nc, in_maps, list(range(NCORES)))
    y = np.concatenate([res.results[c]["y"] for c in range(NCORES)], axis=1)
    return y.astype(np.float32, copy=False)


# revision 22
# speedup vs baseline: 1.0137x; 1.0137x over previous
"""Block-quantized FP8 linear (KLinearFP8) on 8 trn2 NeuronCores.

y[m, n] = sum_k x_dq[m, k] * w_dq[n, k]
  x_dq: per-(row, 128-block) fp8e4m3fn-simulated quantization of x
  w_dq: weight (fp8 values held in fp32) * per-128x128-block scale

Sharding: column-parallel. weight/weight_scale_inv split along N across 8
cores, x replicated; each core computes y[:, c*2048:(c+1)*2048].

Per-core kernel (v2, PE-roofline oriented):
  - w is shipped from host as bf16 (exact: values are fp8-representable),
    already transposed/blocked to the k-on-partitions layout
    wT [128k, KB, NSH] (pure layout permutation, i.e. weight packing).
    On device it is a plain wide DMA into a persistent SBUF tile, in
    4-kb groups split over two queues so matmuls can start early. No
    PE-array transposes and no XBAR use on the weight path.
  - block scales are shipped pre-expanded (bf16, each value repeated
    128x -- pure replication) and staged into SBUF by partition-
    replicate DMAs (stride-0 partition dim), so the on-device scale
    multiply is a unit-stride bf16 tensor_tensor on vector (eligible
    for the DVE 2x perf mode; a stride-0 broadcast operand would force
    1x). Scales are applied to wT in place, per 2-kb group, emitted
    before any matmul that reads the group (Tile follows program
    order). gpsimd is kept to DMA triggers only: its custom-op
    library swaps cost 7-11us, and vector+gpsimd elementwise ops
    contend on SBUF ports, halving both.
  - x path per 128-row m-tile (two K-halves): quantize to TRN fp8e4
    with scale amax/224 (power-of-two rescale of the reference's
    amax/448 grid -> identical rounding), dequantize to bf16, XBAR
    transpose to k-on-partitions. ALL XBAR transposes live on the
    nc.sync queue: the XBAR is a globally exclusive resource --
    concurrent transposes on different queues corrupt each other
    (probed on HW).
  - GEMM: per m-tile, kb outer / 512-col chunk inner; each LDWEIGHTS
    (xT block) is shared by 4 N=512 bf16 matmuls accumulating fp32
    into 4 full PSUM banks; 8-bank rotation double-buffers m-tiles.
  - PSUM evacuation runs entirely on the scalar engine: evac copies
    queued on vector sit behind the next m-tile's quant chain and
    delay the PSUM bank release the next-next m-tile's first matmul
    waits on (~3us PE stall every other m-tile, seen in trace).
"""

import numpy as np

M, K, N = 4096, 4096, 16384
NCORES = 8
NSH = N // NCORES          # 2048 columns of y per core
P = 128
KB = K // P                # 32 k-blocks
KH = KB // 2               # 16 k-blocks per half
MT = M // P                # 32 m-tiles
NB = NSH // P              # 16 n-blocks per core
NCH = NSH // 512           # 4 psum chunks of 512
CHW = 512
FP8_SAFE = 224.0           # 448/2: fits TRN e4m3 (max 240), same rounding grid

_NC_CACHE = {}


def _build(M=M, K=K, NSH=NSH, debug=False):
    import concourse.bass as bass  # noqa: F401
    import concourse.mybir as mybir
    import concourse.tile as tile
    from concourse import bacc

    KB = K // P
    KH = KB // 2
    MT = M // P
    NB = NSH // P
    CHW = min(512, NSH)
    NCH = NSH // CHW
    SG = 2 if KB % 2 == 0 else 1    # k-blocks per scale group
    NSG = KB // SG
    GRP = SG                        # w-load pieces aligned to scale groups
    NG = NSG

    f32, bf16, f8 = mybir.dt.float32, mybir.dt.bfloat16, mybir.dt.float8e4

    nc = bacc.Bacc(None, target_bir_lowering=False, debug=debug)
    x_d = nc.declare_dram_parameter("x", [M, K], f32, isOutput=False)
    # host ships w pre-transposed/blocked: wt[k', kb*NSH + n] = w[n, kb*128+k']
    wt_d = nc.declare_dram_parameter("wt", [P, KB * NSH], bf16, isOutput=False)
    # host ships block scales bf16, expanded 32x: wse[0, (kb*NB+nb)*32+j]
    SE = 32
    wse_d = nc.declare_dram_parameter("wse", [1, KB * NB * SE], bf16,
                                      isOutput=False)
    y_d = nc.declare_dram_parameter("y", [M, NSH], f32, isOutput=True)

    with tile.TileContext(nc) as tc:
        with (
            tc.tile_pool(name="wt", bufs=1) as wtp,
            tc.tile_pool(name="sepool", bufs=2) as sepool,
            tc.tile_pool(name="xrow", bufs=2) as xrp,
            tc.tile_pool(name="xq", bufs=2) as xqp,
            tc.tile_pool(name="xdq", bufs=4) as xdp,
            tc.tile_pool(name="xtp", bufs=7) as xtp,
            tc.tile_pool(name="scales", bufs=4) as spool,
            tc.tile_pool(name="ypool", bufs=4) as ypool,
            tc.tile_pool(name="psum", bufs=8, space="PSUM") as psum,
        ):
            # ---- weight load: plain wide DMAs of the pre-packed layout,
            # all on the scalar queue, each piece overlapping the previous
            # piece's last 128 elements: the WAW dependency chains the
            # transfers, so they stream strictly one at a time, front to
            # back. Without this ALL pieces transfer concurrently and
            # fair-share the DMA engines, starving the small x-row /
            # scale-fill transfers the first matmuls are gated on.
            wT = wtp.tile([P, KB * NSH], bf16)
            for g in range(NG):
                s = g * GRP * NSH - (P if g > 0 else 0)
                e = (g + 1) * GRP * NSH
                nc.scalar.dma_start(wT[:, s:e], wt_d[:, s:e])

            SEW = SG * NB * SE

            def scale_group(g):
                # stage the 32-wide expanded scales via partition-replicate
                # DMA (tiny), then an in-place bf16 multiply on vector whose
                # innermost dims are unit-stride (DVE 2x perf mode; the
                # replicate factor rides a stride-0 MIDDLE dim, which the
                # 2x trigger does not constrain)
                se = sepool.tile([P, SG * NB, SE], bf16, tag="se")
                nc.sync.dma_start(
                    se[:].rearrange("p a b -> p (a b)"),
                    wse_d[:, g * SEW:(g + 1) * SEW].to_broadcast((P, SEW)),
                )
                view = wT[:, g * SG * NSH:(g + 1) * SG * NSH].rearrange(
                    "p (a f x) -> p a f x", a=SG * NB, x=SE
                )
                nc.vector.tensor_tensor(
                    view, view,
                    se[:, :, None, :].to_broadcast((P, SG * NB, P // SE, SE)),
                    mybir.AluOpType.mult,
                )

            def quant_half(mt, kh, splits=1):
                # splits>1 quantizes/transposes in kb-subranges (slice-level
                # ops on full-size tiles) to shorten the time to the first
                # usable xT block (startup only)
                ms = slice(mt * P, (mt + 1) * P)
                xrow = xrp.tile([P, KH, P], f32, tag="xrow")
                sc = spool.tile([P, 3, KH], f32, tag="sc")
                xq = xqp.tile([P, KH, P], f8, tag="xq")
                xdq = xdp.tile([P, KH, P], bf16, tag="xdq")
                xT = xtp.tile([P, KH, P], bf16, tag="xT")
                kq = KH // splits
                for s in range(splits):
                    q = slice(s * kq, (s + 1) * kq)
                    k0 = kh * KH + s * kq
                    ks = slice(k0 * P, (k0 + kq) * P)
                    nc.gpsimd.dma_start(
                        xrow[:, q, :],
                        x_d[ms, ks].rearrange("m (kb x) -> m kb x", x=P),
                    )
                    amax, rinv, s2 = sc[:, 0, q], sc[:, 1, q], sc[:, 2, q]
                    nc.vector.tensor_reduce(
                        amax, xrow[:, q, :], axis=mybir.AxisListType.X,
                        op=mybir.AluOpType.max, apply_absolute_value=True,
                    )
                    nc.vector.reciprocal(rinv, amax)
                    nc.vector.tensor_scalar_mul(rinv, rinv, float(FP8_SAFE))
                    nc.vector.tensor_scalar_mul(s2, amax, float(1.0 / FP8_SAFE))
                    nc.vector.tensor_tensor(
                        xq[:, q, :], xrow[:, q, :],
                        rinv[:, :, None].to_broadcast((P, kq, P)),
                        mybir.AluOpType.mult,
                    )
                    nc.vector.tensor_tensor(
                        xdq[:, q, :], xq[:, q, :],
                        s2[:, :, None].to_broadcast((P, kq, P)),
                        mybir.AluOpType.mult,
                    )
                    nc.sync.dma_start_transpose(
                        xT[:, q, :],
                        xdq[:, q, :].rearrange("p a b -> p (a b)"),
                    )
                return xT

            # Emission order (Tile semantics follow program order): first
            # two scale groups, then mt0+mt1 x pipelines (no wT dep; keeps
            # the PE fed while the remaining scales stream), then the rest
            # of the scales, then the m-loop.
            scale_group(0)
            scale_group(1)
            xT0 = [quant_half(0, 0, splits=2 if KH % 2 == 0 else 1),
                   quant_half(0, 1)]
            xT1 = [quant_half(1, 0), quant_half(1, 1)] if MT > 1 else None
            for g in range(2, NSG):
                scale_group(g)

            for mt in range(MT):
                ms = slice(mt * P, (mt + 1) * P)
                if mt == 0:
                    xThalf = xT0
                elif mt == 1:
                    xThalf = xT1
                else:
                    xThalf = [quant_half(mt, 0), quant_half(mt, 1)]

                pts = [
                    psum.tile([P, CHW], mybir.dt.float32, name=f"pt{c}", tag="pt")
                    for c in range(NCH)
                ]
                for kb in range(KB):
                    kh, kl = divmod(kb, KH)
                    for c in range(NCH):
                        nc.tensor.matmul(
                            pts[c][:],
                            xThalf[kh][:, kl, :],
                            wT[:, kb * NSH + c * CHW:kb * NSH + (c + 1) * CHW],
                            start=(kb == 0),
                            stop=(kb == KB - 1),
                        )
                for c in range(NCH):
                    yt = ypool.tile([P, CHW], mybir.dt.float32, tag="yt")
                    nc.scalar.activation(
                        yt[:], pts[c][:],
                        mybir.ActivationFunctionType.Copy,
                    )
                    nc.gpsimd.dma_start(y_d[ms, c * CHW:(c + 1) * CHW], yt[:])

    nc.compile()
    return nc


def _prep_inputs(x, weight, weight_scale_inv):
    import ml_dtypes

    x = np.ascontiguousarray(np.asarray(x, dtype=np.float32))
    weight = np.asarray(weight, dtype=np.float32)
    ws = np.asarray(weight_scale_inv, dtype=np.float32)
    KBl = weight.shape[1] // P
    nshard = weight.shape[0] // NCORES
    in_maps = []
    for c in range(NCORES):
        wsh = weight[c * nshard:(c + 1) * nshard].astype(ml_dtypes.bfloat16)
        # pack: wt[k', kb, n] = w[n, kb*128+k']  -> flat [128, KB*NSH]
        wt = np.ascontiguousarray(
            wsh.reshape(nshard, KBl, P).transpose(2, 1, 0)
        ).reshape(P, KBl * nshard)
        nbsh = nshard // P
        # expanded bf16 block scales: [kb, nb] -> repeat each value 32x
        wse = np.repeat(
            np.ascontiguousarray(
                ws[c * nbsh:(c + 1) * nbsh].T
            ).astype(ml_dtypes.bfloat16).reshape(-1),
            32,
        )[None, :]
        in_maps.append({"x": x, "wt": wt, "wse": wse})
    return in_maps


def kernel(x, weight, weight_scale_inv):
    from concourse.bass_utils import run_bass_kernel_spmd

    if "nc" not in _NC_CACHE:
        _NC_CACHE["nc"] = _build()
    nc = _NC_CACHE["nc"]

    in_maps = _prep_inputs(x, weight, weight_scale_inv)
    res = run_bass_kernel_spmd(nc, in_maps, list(range(NCORES)))
    y = np.concatenate([res.results[c]["y"] for c in range(NCORES)], axis=1)
    return y.astype(np.float32, copy=False)
